# revision 4
# baseline (speedup 1.0000x reference)
"""Trainium2 Bass kernel for nn_Antecedents: fuzzy-rule antecedent activations.

Computes out[n, r] = prod_v memberships[v, n, set_v(r)] over the full
Cartesian product of fuzzy sets (R = 4**6 = 4096 rules), for N = 16384
samples, data-parallel over 8 NeuronCores (2048 samples per core).

All-log-space hybrid: the host feeds, per core, a transposed duplicate
of the memberships mT48[(row), n] f32 with rows laid out as
[v1..v5 | v1..v5 dup | v0 | v0 dup] x 4 sets, plus the matching one-hot
matrix O48[row, r] bf16.  On device:

  ln48 = Ln(mT48)                    (one ACT op)
  LC   = bf16(ln48); LC[lo rows] = ln48 - LC   (hi/lo split, DVE)

so LC[0:40] = [v1-5 hi | v1-5 lo] and LC[40:48] = [v0 hi | v0 lo].
Per j (sample n = p*16 + j), lhsT = LC[:, j::16]:

 * B-js: PE computes S15 = lhsT[0:40].T @ O48[0:40, 0:1024] (2 matmuls,
   K=40 -> [128, 1024] PSUM), ACT drains with one Exp -> e1024 bf16,
   DVE broadcasts x X0[s0] (4 tensor_scalar ops) into the 4096-wide
   output block.  Engine cost ~1.4us PE / ~1.25us ACT / ~2us DVE.

 * C-js: PE computes the full S = lhsT.T @ O48 (8 matmuls, K=48), ACT
   drains each [128, 2048] PSUM half with one Exp straight into the
   output tile.  ~4.2us ACT, zero DVE.

The hi/lo split keeps the log-sum at f32-level accuracy through the
bf16 PE datapath; output is bf16 (max rel err ~5e-3 vs the 2e-2 gate).
Output DMAs alternate between the Sync and GpSimd queues so neither
issue queue becomes the bottleneck.
"""

import numpy as np
from contextlib import ExitStack

import concourse.bass as bass
import concourse.tile as tile
from concourse import bacc, mybir
from concourse.bass_utils import run_bass_kernel_spmd

N_VARS = 6
N_FULL = 16384
N_SETS = 4
N_CORES = 8
N_SHARD = N_FULL // N_CORES  # 2048
P = 128
J = N_SHARD // P             # 16 samples per partition
R = N_SETS ** N_VARS         # 4096
F32 = mybir.dt.float32
BF16 = mybir.dt.bfloat16
MUL = mybir.AluOpType.mult
SUB = mybir.AluOpType.subtract
EXP = mybir.ActivationFunctionType.Exp
LN = mybir.ActivationFunctionType.Ln

C_JS = (5, 10, 15)  # full-matmul js (ACT-drained); rest are B-js
KK = 56             # lhsT rows: [hi(24) | pad(8) | lo(24)], pad rows are ln(1)=0

LAST_RESULTS = None
_CACHE = {}


def _bap(tile_ap, col_off, dims, nparts=None):
    base = tile_ap[:]
    p0 = base.ap[0] if nparts is None else [base.ap[0][0], nparts]
    return bass.AP(
        tensor=base.tensor,
        offset=base.offset + col_off,
        ap=[p0] + [[s, c] for (s, c) in dims],
    )


def _pslice(tile_ap, p0, p1):
    """Partition-sliced full-width AP of a tile."""
    return tile_ap[p0:p1, :]


def build_nc():
    nc = bacc.Bacc(
        "TRN2", target_bir_lowering=False, debug=False, num_devices=N_CORES
    )
    m = nc.dram_tensor(
        "memberships", [N_VARS, N_SHARD, N_SETS], F32, kind="ExternalInput"
    ).ap()
    mT = nc.dram_tensor("mT64", [64, N_SHARD], F32, kind="ExternalInput").ap()
    ohc = nc.dram_tensor("ohc", [KK, R], BF16, kind="ExternalInput").ap()
    ohb = nc.dram_tensor("ohb", [KK, 1024], BF16, kind="ExternalInput").ap()
    out = nc.dram_tensor("out", [N_SHARD, R], BF16, kind="ExternalOutput").ap()
    out_v = out.rearrange("(p f) r -> p (f r)", p=P)  # [128, J*R]

    with tile.TileContext(nc) as tc, ExitStack() as ctx:
        xpool = ctx.enter_context(tc.tile_pool(name="x", bufs=1))
        spool = ctx.enter_context(tc.tile_pool(name="scratch", bufs=3))
        o1pool = ctx.enter_context(tc.tile_pool(name="o1", bufs=4))
        pbpool = ctx.enter_context(tc.psum_pool(name="pb", bufs=2))
        pcpool = ctx.enter_context(tc.psum_pool(name="pc", bufs=1))

        # ---- input DMAs --------------------------------------------------
        x0 = xpool.tile([P, J * N_SETS], F32, tag="x0")
        nc.sync.dma_start(
            out=x0[:], in_=m[0].rearrange("(p f) s -> p (f s)", p=P)
        )
        mt = xpool.tile([64, N_SHARD], F32, tag="mt")
        nc.sync.dma_start(out=mt[:], in_=mT)
        ohC = xpool.tile([KK, R], BF16, tag="ohc")
        nc.sync.dma_start(out=ohC[:], in_=ohc)
        ohB = xpool.tile([KK, 1024], BF16, tag="ohb")
        nc.sync.dma_start(out=ohB[:], in_=ohb)

        # ---- log + hi/lo split -------------------------------------------
        ln64 = xpool.tile([64, N_SHARD], F32, tag="ln64")
        nc.scalar.activation(ln64[:], mt[:], LN)
        LC = xpool.tile([64, N_SHARD], BF16, tag="LC")
        nc.vector.tensor_copy(LC[:], ln64[:])
        # lo rows overwrite their hi copy in place: lo = ln - bf16(ln)
        nc.vector.tensor_tensor(
            out=LC[32:56, :],
            in0=ln64[32:56, :],
            in1=LC[32:56, :],
            op=SUB,
        )

        def x0c(j, s):
            c = j * N_SETS + s
            return x0[:, c : c + 1]

        def emit_b_j(j):
            lhsT = _bap(LC, j, [(J, P)], nparts=KK)
            ps = pbpool.tile([P, 1024], F32, tag="psb")
            for c in range(2):
                nc.tensor.matmul(
                    out=ps[:, c * 512 : (c + 1) * 512],
                    lhsT=lhsT,
                    rhs=ohB[:, c * 512 : (c + 1) * 512],
                    start=True,
                    stop=True,
                )
            e1024 = spool.tile([P, 1024], BF16, tag="e1024")
            nc.scalar.activation(e1024[:], ps[:], EXP)
            ot = o1pool.tile([P, R], BF16, tag="o1")
            for s in range(N_SETS):
                nc.vector.tensor_scalar_mul(
                    ot[:, 1024 * s : 1024 * (s + 1)],
                    e1024[:],
                    x0c(j, s),
                )
            for c in range(2):
                eng = nc.sync if (j + c) % 2 == 0 else nc.gpsimd
                eng.dma_start(
                    out=out_v[:, j * R + c * 2048 : j * R + (c + 1) * 2048],
                    in_=ot[:, c * 2048 : (c + 1) * 2048],
                )

        def emit_c_j(j):
            lhsT = _bap(LC, j, [(J, P)], nparts=KK)
            ot = o1pool.tile([P, R], BF16, tag="o1")
            for h in range(2):
                ps = pcpool.tile([P, 2048], F32, tag="psc")
                for c in range(4):
                    col = h * 2048 + c * 512
                    nc.tensor.matmul(
                        out=ps[:, c * 512 : (c + 1) * 512],
                        lhsT=lhsT,
                        rhs=ohC[:, col : col + 512],
                        start=True,
                        stop=True,
                    )
                nc.scalar.activation(
                    ot[:, h * 2048 : (h + 1) * 2048], ps[:], EXP
                )
                eng = nc.sync if (j + h) % 2 == 0 else nc.gpsimd
                eng.dma_start(
                    out=out_v[:, j * R + h * 2048 : j * R + (h + 1) * 2048],
                    in_=ot[:, h * 2048 : (h + 1) * 2048],
                )

        for j in range(J):
            if j in C_JS:
                emit_c_j(j)
            else:
                emit_b_j(j)

    nc.compile()
    return nc


def _get_nc():
    if "nc" not in _CACHE:
        _CACHE["nc"] = build_nc()
    return _CACHE["nc"]


def _onehots():
    """(ohc [56, R], ohb [56, 1024]) bf16 matching LC rows
    [v0..v5 hi (24) | pad (8) | v0..v5 lo (24)]."""
    import ml_dtypes

    r = np.arange(R)
    o24 = np.zeros((24, R), dtype=np.float32)
    for v in range(N_VARS):
        sv = (r >> (2 * (N_VARS - 1 - v))) & 3
        for s in range(N_SETS):
            o24[v * N_SETS + s] = (sv == s).astype(np.float32)
    pad = np.zeros((8, R), dtype=np.float32)
    ohc = np.concatenate([o24, pad, o24], axis=0)
    # B variant: v0 rows zeroed, only first 1024 columns
    o24b = o24.copy()
    o24b[0:N_SETS] = 0.0
    ohb = np.concatenate([o24b, pad, o24b], axis=0)[:, 0:1024]
    return ohc.astype(ml_dtypes.bfloat16), np.ascontiguousarray(
        ohb.astype(ml_dtypes.bfloat16)
    )


def _mt64(shard: np.ndarray) -> np.ndarray:
    """[64, N_SHARD] f32: rows [v0..v5 | ones(8) | v0..v5 | ones(8)]."""
    t = shard.transpose(0, 2, 1).reshape(N_VARS * N_SETS, N_SHARD)  # [(v,s), n]
    ones = np.ones((8, N_SHARD), dtype=np.float32)
    full = np.concatenate([t, ones, t, ones], axis=0)
    return np.ascontiguousarray(np.maximum(full, 1e-38))


def kernel(memberships):
    global LAST_RESULTS
    m = np.ascontiguousarray(np.asarray(memberships, dtype=np.float32))
    assert m.shape == (N_VARS, N_FULL, N_SETS), m.shape
    nc = _get_nc()
    ohc, ohb = _onehots()
    shards = np.split(m, N_CORES, axis=1)
    in_maps = [
        {
            "memberships": np.ascontiguousarray(s),
            "mT64": _mt64(s),
            "ohc": ohc,
            "ohb": ohb,
        }
        for s in shards
    ]
    res = run_bass_kernel_spmd(nc, in_maps, core_ids=list(range(N_CORES)))
    LAST_RESULTS = res
    return np.concatenate(
        [res.results[i]["out"] for i in range(N_CORES)], axis=0
    ).astype(np.float32)


# revision 6
# speedup vs baseline: 1.0734x; 1.0734x over previous
"""Trainium2 Bass kernel for nn_Antecedents: fuzzy-rule antecedent activations.

Computes out[n, r] = prod_v memberships[v, n, set_v(r)] over the full
Cartesian product of fuzzy sets (R = 4**6 = 4096 rules), for N = 16384
samples, data-parallel over 8 NeuronCores (2048 samples per core).

Log-space hybrid.  The host feeds, per core, a transposed j-major copy
of the memberships mT64[row, j*128+m] f32 (row layout
[v0..v5 hi (24) | ones (8) | v0..v5 dup (24) | ones (8)], sample
n = m*16+j) plus matching one-hot matrices ohc[56, 4096] / ohb[56,1024]
bf16 (ohb has the v0 rows zeroed).  On device:

  ln64 = Ln(mT64)          (ACT, chunked 512+1536 cols to cut latency)
  LC   = bf16(ln64); LC[32:56] = ln64[32:56] - LC[32:56]   (hi/lo, DVE)

Per j, lhsT = LC[0:56, j*128:(j+1)*128] (contiguous):

 * B-js (paired): PE computes S15 = lhsT.T @ ohb for two js into one
   [128, 2048] PSUM tile (4 matmuls, K=56), ACT drains it with a single
   Exp -> e2048 bf16, DVE broadcasts x X0[s0] (4 tensor_scalar per j)
   into the 4096-wide output blocks.

 * C-js: PE computes the full S = lhsT.T @ ohc (8 matmuls), ACT drains
   each [128, 2048] PSUM half with one Exp straight into the output
   tile; zero DVE work.

The hi/lo split keeps the log-sum at f32-level accuracy through the
bf16 PE datapath; output is bf16 (max rel err ~8e-3 vs the 2e-2 gate).
"""

import numpy as np
from contextlib import ExitStack

import concourse.bass as bass
import concourse.tile as tile
from concourse import bacc, mybir
from concourse.bass_utils import run_bass_kernel_spmd

N_VARS = 6
N_FULL = 16384
N_SETS = 4
N_CORES = 8
N_SHARD = N_FULL // N_CORES  # 2048
P = 128
J = N_SHARD // P             # 16 samples per partition
R = N_SETS ** N_VARS         # 4096
F32 = mybir.dt.float32
BF16 = mybir.dt.bfloat16
MUL = mybir.AluOpType.mult
SUB = mybir.AluOpType.subtract
EXP = mybir.ActivationFunctionType.Exp
LN = mybir.ActivationFunctionType.Ln

KK = 56  # lhsT rows: [hi(24) | pad(8) | lo(24)]
C_JS = (5, 10, 15)
# B-pairs + one single (13 B js), interleaved with C js in emission order
SCHEDULE = [
    ("pair", 0, 1),
    ("pair", 2, 3),
    ("pair", 4, 6),
    ("c", 5),
    ("pair", 7, 8),
    ("pair", 9, 11),
    ("c", 10),
    ("pair", 12, 13),
    ("single", 14),
    ("c", 15),
]
PREP0 = 512  # columns (4 js) prepared before the first matmul

LAST_RESULTS = None
_CACHE = {}


def build_nc():
    nc = bacc.Bacc(
        "TRN2", target_bir_lowering=False, debug=False, num_devices=N_CORES
    )
    m = nc.dram_tensor(
        "memberships", [N_VARS, N_SHARD, N_SETS], F32, kind="ExternalInput"
    ).ap()
    mT = nc.dram_tensor("mT64", [64, N_SHARD], F32, kind="ExternalInput").ap()
    ohc = nc.dram_tensor("ohc", [KK, R], BF16, kind="ExternalInput").ap()
    ohb = nc.dram_tensor("ohb", [KK, 1024], BF16, kind="ExternalInput").ap()
    out = nc.dram_tensor("out", [N_SHARD, R], BF16, kind="ExternalOutput").ap()
    out_v = out.rearrange("(p f) r -> p (f r)", p=P)  # [128, J*R]

    with tile.TileContext(nc) as tc, ExitStack() as ctx:
        xpool = ctx.enter_context(tc.tile_pool(name="x", bufs=1))
        spool = ctx.enter_context(tc.tile_pool(name="scratch", bufs=3))
        o1pool = ctx.enter_context(tc.tile_pool(name="o1", bufs=4))
        ppool = ctx.enter_context(tc.psum_pool(name="pp", bufs=2))

        # ---- ACT warm-up: settle the ln/exp table load off the critical path
        warm = xpool.tile([P, 8], F32, tag="warm")
        nc.gpsimd.memset(warm[:], 1.0)
        nc.scalar.activation(warm[:, 0:1], warm[:, 1:2], LN)
        nc.scalar.activation(warm[:, 2:3], warm[:, 3:4], EXP)

        # ---- input DMAs --------------------------------------------------
        mt = xpool.tile([64, N_SHARD], F32, tag="mt")
        nc.sync.dma_start(out=mt[:, 0:PREP0], in_=mT[:, 0:PREP0])
        nc.sync.dma_start(out=mt[:, PREP0:], in_=mT[:, PREP0:])
        x0 = xpool.tile([P, J * N_SETS], F32, tag="x0")
        nc.sync.dma_start(
            out=x0[:], in_=m[0].rearrange("(p f) s -> p (f s)", p=P)
        )
        ohC = xpool.tile([KK, R], BF16, tag="ohc")
        nc.gpsimd.dma_start(out=ohC[:], in_=ohc)
        ohB = xpool.tile([KK, 1024], BF16, tag="ohb")
        nc.gpsimd.dma_start(out=ohB[:], in_=ohb)

        # ---- log + hi/lo split (chunked) ---------------------------------
        ln64 = xpool.tile([64, N_SHARD], F32, tag="ln64")
        LC = xpool.tile([64, N_SHARD], BF16, tag="LC")

        def prep(c0, c1):
            nc.scalar.activation(ln64[:, c0:c1], mt[:, c0:c1], LN)
            nc.vector.tensor_copy(LC[:, c0:c1], ln64[:, c0:c1])
            nc.vector.tensor_tensor(
                out=LC[32:56, c0:c1],
                in0=ln64[32:56, c0:c1],
                in1=LC[32:56, c0:c1],
                op=SUB,
            )

        prep(0, PREP0)

        def x0c(j, s):
            c = j * N_SETS + s
            return x0[:, c : c + 1]

        def lhsT(j):
            return LC[0:KK, j * P : (j + 1) * P]

        def final_and_ship(j, e_ap):
            ot = o1pool.tile([P, R], BF16, tag="o1")
            for s in range(N_SETS):
                nc.vector.tensor_scalar_mul(
                    ot[:, 1024 * s : 1024 * (s + 1)], e_ap, x0c(j, s)
                )
            nc.sync.dma_start(
                out=out_v[:, j * R : (j + 1) * R], in_=ot[:]
            )

        def emit_pair(ja, jb):
            ps = ppool.tile([P, 2048], F32, tag="ps")
            for idx, j in enumerate((ja, jb)):
                for c in range(2):
                    col = idx * 1024 + c * 512
                    nc.tensor.matmul(
                        out=ps[:, col : col + 512],
                        lhsT=lhsT(j),
                        rhs=ohB[:, c * 512 : (c + 1) * 512],
                        start=True,
                        stop=True,
                    )
            e2048 = spool.tile([P, 2048], BF16, tag="e2048")
            nc.scalar.activation(e2048[:], ps[:], EXP)
            final_and_ship(ja, e2048[:, 0:1024])
            final_and_ship(jb, e2048[:, 1024:2048])

        def emit_single(j):
            ps = ppool.tile([P, 2048], F32, tag="ps")
            for c in range(2):
                nc.tensor.matmul(
                    out=ps[:, c * 512 : (c + 1) * 512],
                    lhsT=lhsT(j),
                    rhs=ohB[:, c * 512 : (c + 1) * 512],
                    start=True,
                    stop=True,
                )
            e2048 = spool.tile([P, 2048], BF16, tag="e2048")
            nc.scalar.activation(e2048[:, 0:1024], ps[:, 0:1024], EXP)
            final_and_ship(j, e2048[:, 0:1024])

        def emit_c(j):
            ot = o1pool.tile([P, R], BF16, tag="o1")
            for h in range(2):
                ps = ppool.tile([P, 2048], F32, tag="ps")
                for c in range(4):
                    col = h * 2048 + c * 512
                    nc.tensor.matmul(
                        out=ps[:, c * 512 : (c + 1) * 512],
                        lhsT=lhsT(j),
                        rhs=ohC[:, col : col + 512],
                        start=True,
                        stop=True,
                    )
                nc.scalar.activation(
                    ot[:, h * 2048 : (h + 1) * 2048], ps[:], EXP
                )
            nc.sync.dma_start(
                out=out_v[:, j * R : (j + 1) * R], in_=ot[:]
            )

        done_rest_prep = False
        for step in SCHEDULE:
            if step[0] == "pair":
                emit_pair(step[1], step[2])
            elif step[0] == "single":
                emit_single(step[1])
            else:
                emit_c(step[1])
            if not done_rest_prep:
                prep(PREP0, N_SHARD)
                done_rest_prep = True

    nc.compile()
    return nc


def _get_nc():
    if "nc" not in _CACHE:
        _CACHE["nc"] = build_nc()
    return _CACHE["nc"]


def _onehots():
    """(ohc [56, R], ohb [56, 1024]) bf16 matching LC rows
    [v0..v5 hi (24) | pad (8) | v0..v5 lo (24)]."""
    import ml_dtypes

    r = np.arange(R)
    o24 = np.zeros((24, R), dtype=np.float32)
    for v in range(N_VARS):
        sv = (r >> (2 * (N_VARS - 1 - v))) & 3
        for s in range(N_SETS):
            o24[v * N_SETS + s] = (sv == s).astype(np.float32)
    pad = np.zeros((8, R), dtype=np.float32)
    ohc = np.concatenate([o24, pad, o24], axis=0)
    o24b = o24.copy()
    o24b[0:N_SETS] = 0.0
    ohb = np.concatenate([o24b, pad, o24b], axis=0)[:, 0:1024]
    return ohc.astype(ml_dtypes.bfloat16), np.ascontiguousarray(
        ohb.astype(ml_dtypes.bfloat16)
    )


def _mt64(shard: np.ndarray) -> np.ndarray:
    """[64, N_SHARD] f32, j-major columns (col j*128+m = sample m*16+j),
    rows [v0..v5 | ones(8) | v0..v5 | ones(8)]."""
    t = shard.transpose(0, 2, 1).reshape(N_VARS * N_SETS, N_SHARD)  # [(v,s), n]
    ones = np.ones((8, N_SHARD), dtype=np.float32)
    full = np.concatenate([t, ones, t, ones], axis=0)
    full = np.maximum(full, 1e-38)
    # n = m*16 + j  ->  column j*128 + m
    full = full.reshape(64, P, J).transpose(0, 2, 1).reshape(64, N_SHARD)
    return np.ascontiguousarray(full)


def kernel(memberships):
    global LAST_RESULTS
    m = np.ascontiguousarray(np.asarray(memberships, dtype=np.float32))
    assert m.shape == (N_VARS, N_FULL, N_SETS), m.shape
    nc = _get_nc()
    ohc, ohb = _onehots()
    shards = np.split(m, N_CORES, axis=1)
    in_maps = [
        {
            "memberships": np.ascontiguousarray(s),
            "mT64": _mt64(s),
            "ohc": ohc,
            "ohb": ohb,
        }
        for s in shards
    ]
    res = run_bass_kernel_spmd(nc, in_maps, core_ids=list(range(N_CORES)))
    LAST_RESULTS = res
    return np.concatenate(
        [res.results[i]["out"] for i in range(N_CORES)], axis=0
    ).astype(np.float32)


# revision 7
# speedup vs baseline: 1.2268x; 1.1429x over previous
"""Trainium2 Bass kernel for nn_Antecedents: fuzzy-rule antecedent activations.

Computes out[n, r] = prod_v memberships[v, n, set_v(r)] over the full
Cartesian product of fuzzy sets (R = 4**6 = 4096 rules), for N = 16384
samples, data-parallel over 8 NeuronCores (2048 samples per core).

Log-space hybrid.  The host feeds, per core, a transposed j-major copy
of the memberships mT64[row, j*128+m] f32 (row layout
[v0..v5 hi (24) | ones (8) | v0..v5 dup (24) | ones (8)], sample
n = m*16+j) plus matching one-hot matrices ohc[56, 4096] / ohb[56,1024]
bf16 (ohb has the v0 rows zeroed).  On device:

  ln64 = Ln(mT64)          (ACT, chunked 512+1536 cols to cut latency)
  LC   = bf16(ln64); LC[32:56] = ln64[32:56] - LC[32:56]   (hi/lo, DVE)

Per j, lhsT = LC[0:56, j*128:(j+1)*128] (contiguous):

 * B-js (paired): PE computes S15 = lhsT.T @ ohb for two js into one
   [128, 2048] PSUM tile (4 matmuls, K=56), ACT drains it with a single
   Exp -> e2048 bf16, DVE broadcasts x X0[s0] (4 tensor_scalar per j)
   into the 4096-wide output blocks.

 * C-js: PE computes the full S = lhsT.T @ ohc (8 matmuls), ACT drains
   each [128, 2048] PSUM half with one Exp straight into the output
   tile; zero DVE work.

The hi/lo split keeps the log-sum at f32-level accuracy through the
bf16 PE datapath; output is bf16 (max rel err ~8e-3 vs the 2e-2 gate).
"""

import numpy as np
from contextlib import ExitStack

import concourse.bass as bass
import concourse.tile as tile
from concourse import bacc, mybir
from concourse.bass_utils import run_bass_kernel_spmd

N_VARS = 6
N_FULL = 16384
N_SETS = 4
N_CORES = 8
N_SHARD = N_FULL // N_CORES  # 2048
P = 128
J = N_SHARD // P             # 16 samples per partition
R = N_SETS ** N_VARS         # 4096
F32 = mybir.dt.float32
BF16 = mybir.dt.bfloat16
MUL = mybir.AluOpType.mult
SUB = mybir.AluOpType.subtract
EXP = mybir.ActivationFunctionType.Exp
LN = mybir.ActivationFunctionType.Ln

KK = 56  # lhsT rows: [hi(24) | pad(8) | lo(24)]
# B-pairs interleaved with C js; C js mid-stream so the kernel ends on
# short pair tails, not an 8-matmul C chain.
SCHEDULE = [
    ("pair", 0, 1),
    ("pair", 2, 3),
    ("c", 4),
    ("pair", 5, 6),
    ("pair", 7, 8),
    ("c", 9),
    ("pair", 10, 11),
    ("pair", 12, 13),
    ("pair", 14, 15),
]
PREP0 = 512  # columns (4 js) prepared before the first matmul

LAST_RESULTS = None
_CACHE = {}


def build_nc():
    nc = bacc.Bacc(
        "TRN2", target_bir_lowering=False, debug=False, num_devices=N_CORES
    )
    m = nc.dram_tensor(
        "memberships", [N_VARS, N_SHARD, N_SETS], F32, kind="ExternalInput"
    ).ap()
    mT = nc.dram_tensor("mT64", [64, N_SHARD], F32, kind="ExternalInput").ap()
    ohc = nc.dram_tensor("ohc", [KK, R], BF16, kind="ExternalInput").ap()
    ohb = nc.dram_tensor("ohb", [KK, 1024], BF16, kind="ExternalInput").ap()
    out = nc.dram_tensor("out", [N_SHARD, R], BF16, kind="ExternalOutput").ap()
    out_v = out.rearrange("(p f) r -> p (f r)", p=P)  # [128, J*R]

    with tile.TileContext(nc) as tc, ExitStack() as ctx:
        xpool = ctx.enter_context(tc.tile_pool(name="x", bufs=1))
        spool = ctx.enter_context(tc.tile_pool(name="scratch", bufs=3))
        o1pool = ctx.enter_context(tc.tile_pool(name="o1", bufs=4))
        ppool = ctx.enter_context(tc.psum_pool(name="pp", bufs=2))

        warm = xpool.tile([P, 8], F32, tag="warm")
        nc.gpsimd.memset(warm[:], 1.0)

        # ---- input DMAs --------------------------------------------------
        mt = xpool.tile([64, N_SHARD], F32, tag="mt")
        nc.sync.dma_start(out=mt[:, 0:PREP0], in_=mT[:, 0:PREP0])
        nc.sync.dma_start(out=mt[:, PREP0:], in_=mT[:, PREP0:])
        x0 = xpool.tile([P, J * N_SETS], F32, tag="x0")
        nc.sync.dma_start(
            out=x0[:], in_=m[0].rearrange("(p f) s -> p (f s)", p=P)
        )
        ohC = xpool.tile([KK, R], BF16, tag="ohc")
        nc.gpsimd.dma_start(out=ohC[:], in_=ohc)
        ohB = xpool.tile([KK, 1024], BF16, tag="ohb")
        nc.gpsimd.dma_start(out=ohB[:], in_=ohb)

        # ---- log + hi/lo split (chunked) ---------------------------------
        ln64 = xpool.tile([64, N_SHARD], F32, tag="ln64")
        LC = xpool.tile([64, N_SHARD], BF16, tag="LC")

        def prep(c0, c1):
            nc.scalar.activation(ln64[:, c0:c1], mt[:, c0:c1], LN)
            nc.vector.tensor_copy(LC[:, c0:c1], ln64[:, c0:c1])
            nc.vector.tensor_tensor(
                out=LC[32:56, c0:c1],
                in0=ln64[32:56, c0:c1],
                in1=LC[32:56, c0:c1],
                op=SUB,
            )

        prep(0, PREP0)
        prep(PREP0, N_SHARD)
        # dummy Exp: pulls the ln->exp table switch off the critical path
        # (overlaps the first pair's matmuls)
        nc.scalar.activation(warm[:, 2:3], warm[:, 3:4], EXP)

        def x0c(j, s):
            c = j * N_SETS + s
            return x0[:, c : c + 1]

        def lhsT(j):
            return LC[0:KK, j * P : (j + 1) * P]

        def final_and_ship(j, e_ap):
            ot = o1pool.tile([P, R], BF16, tag="o1")
            for s in range(N_SETS):
                nc.vector.tensor_scalar_mul(
                    ot[:, 1024 * s : 1024 * (s + 1)], e_ap, x0c(j, s)
                )
            nc.sync.dma_start(
                out=out_v[:, j * R : (j + 1) * R], in_=ot[:]
            )

        def emit_pair(ja, jb):
            ps = ppool.tile([P, 2048], F32, tag="ps")
            for idx, j in enumerate((ja, jb)):
                for c in range(2):
                    col = idx * 1024 + c * 512
                    nc.tensor.matmul(
                        out=ps[:, col : col + 512],
                        lhsT=lhsT(j),
                        rhs=ohB[:, c * 512 : (c + 1) * 512],
                        start=True,
                        stop=True,
                    )
            e2048 = spool.tile([P, 2048], BF16, tag="e2048")
            nc.scalar.activation(e2048[:], ps[:], EXP)
            final_and_ship(ja, e2048[:, 0:1024])
            final_and_ship(jb, e2048[:, 1024:2048])

        def emit_single(j):
            ps = ppool.tile([P, 2048], F32, tag="ps")
            for c in range(2):
                nc.tensor.matmul(
                    out=ps[:, c * 512 : (c + 1) * 512],
                    lhsT=lhsT(j),
                    rhs=ohB[:, c * 512 : (c + 1) * 512],
                    start=True,
                    stop=True,
                )
            e2048 = spool.tile([P, 2048], BF16, tag="e2048")
            nc.scalar.activation(e2048[:, 0:1024], ps[:, 0:1024], EXP)
            final_and_ship(j, e2048[:, 0:1024])

        def emit_c(j):
            ot = o1pool.tile([P, R], BF16, tag="o1")
            for h in range(2):
                ps = ppool.tile([P, 2048], F32, tag="ps")
                for c in range(4):
                    col = h * 2048 + c * 512
                    nc.tensor.matmul(
                        out=ps[:, c * 512 : (c + 1) * 512],
                        lhsT=lhsT(j),
                        rhs=ohC[:, col : col + 512],
                        start=True,
                        stop=True,
                    )
                nc.scalar.activation(
                    ot[:, h * 2048 : (h + 1) * 2048], ps[:], EXP
                )
            nc.sync.dma_start(
                out=out_v[:, j * R : (j + 1) * R], in_=ot[:]
            )

        for step in SCHEDULE:
            if step[0] == "pair":
                emit_pair(step[1], step[2])
            elif step[0] == "single":
                emit_single(step[1])
            else:
                emit_c(step[1])

    nc.compile()
    return nc


def _get_nc():
    if "nc" not in _CACHE:
        _CACHE["nc"] = build_nc()
    return _CACHE["nc"]


def _onehots():
    """(ohc [56, R], ohb [56, 1024]) bf16 matching LC rows
    [v0..v5 hi (24) | pad (8) | v0..v5 lo (24)]."""
    import ml_dtypes

    r = np.arange(R)
    o24 = np.zeros((24, R), dtype=np.float32)
    for v in range(N_VARS):
        sv = (r >> (2 * (N_VARS - 1 - v))) & 3
        for s in range(N_SETS):
            o24[v * N_SETS + s] = (sv == s).astype(np.float32)
    pad = np.zeros((8, R), dtype=np.float32)
    ohc = np.concatenate([o24, pad, o24], axis=0)
    o24b = o24.copy()
    o24b[0:N_SETS] = 0.0
    ohb = np.concatenate([o24b, pad, o24b], axis=0)[:, 0:1024]
    return ohc.astype(ml_dtypes.bfloat16), np.ascontiguousarray(
        ohb.astype(ml_dtypes.bfloat16)
    )


def _mt64(shard: np.ndarray) -> np.ndarray:
    """[64, N_SHARD] f32, j-major columns (col j*128+m = sample m*16+j),
    rows [v0..v5 | ones(8) | v0..v5 | ones(8)]."""
    t = shard.transpose(0, 2, 1).reshape(N_VARS * N_SETS, N_SHARD)  # [(v,s), n]
    ones = np.ones((8, N_SHARD), dtype=np.float32)
    full = np.concatenate([t, ones, t, ones], axis=0)
    full = np.maximum(full, 1e-38)
    # n = m*16 + j  ->  column j*128 + m
    full = full.reshape(64, P, J).transpose(0, 2, 1).reshape(64, N_SHARD)
    return np.ascontiguousarray(full)


def kernel(memberships):
    global LAST_RESULTS
    m = np.ascontiguousarray(np.asarray(memberships, dtype=np.float32))
    assert m.shape == (N_VARS, N_FULL, N_SETS), m.shape
    nc = _get_nc()
    ohc, ohb = _onehots()
    shards = np.split(m, N_CORES, axis=1)
    in_maps = [
        {
            "memberships": np.ascontiguousarray(s),
            "mT64": _mt64(s),
            "ohc": ohc,
            "ohb": ohb,
        }
        for s in shards
    ]
    res = run_bass_kernel_spmd(nc, in_maps, core_ids=list(range(N_CORES)))
    LAST_RESULTS = res
    return np.concatenate(
        [res.results[i]["out"] for i in range(N_CORES)], axis=0
    ).astype(np.float32)
